# revision 31
# baseline (speedup 1.0000x reference)
"""Trainium2 Bass kernel for nn_GatherModel (NNConv GNN message passing).

8-core SPMD, edge-parallel sharded by destination node block:
  - core k owns nodes [k*6250, (k+1)*6250) and all edges whose dst lies there
  - per-edge weight matrices W'_e (o-major, WITHOUT the en2 bias) are built
    once on device (PE, bf16) and streamed bf16 from HBM each of the 6 steps
  - per-edge contraction msg = x_src @ W'_e runs on the Vector engine via a
    hand-built SEGMENTED multiply+scan DVE op (bf16 in/out, scan resets each
    42-element page) with a hand-written 2x_1port uop variant (two packed
    bf16 elements per cycle, ~1.07us per 128x1764 tile) — segment ends are
    the per-o sums, read directly as a strided lhsT by the scatter matmul
  - the en2 bias contribution is applied at the aggregate level:
    aggr_bias[o, v] = B^T @ XS where XS[i, v] = sum_e x[e, i] * onehot[e, v]
    is accumulated on the PE alongside the message scatter
  - scatter (segment-sum over dst) is a PE matmul against on-device-built
    bf16 one-hot window matrices; node update runs fp32 in transposed
    feature layout
  - each step ends with an 8-core AllGather of bf16 node features
"""
import dataclasses
import numpy as np
import ml_dtypes

import concourse.bacc as bacc
import concourse.bass as bass
import concourse.bass_isa as bass_isa
import concourse.mybir as mybir
import concourse.tile as tile
from concourse import bass_utils, dve_ops
from concourse.dve_spec import Spec, Src0, Src1, scan, AluOp, lower, _has_src1
from concourse.dve_uop import (DveOpSpec, Trigger, AluInp, DelayInp, InpSel,
                               OutSel, OutPath, UopConfig, UopDpConfig,
                               ENABLE, DISABLE)

N = 50000
E = 150000
D_IN = 42
D_H = 42
E_IN = 10
E_H = 128
STEPS = 6
N_CORES = 8
NPC = N // N_CORES          # 6250 nodes per core
WIN = 128                   # scatter window (node block) size
N_WIN = (NPC + WIN - 1) // WIN  # 49 windows per core, last partial (106)
NW = D_H * D_H              # 1764
F32 = mybir.dt.float32
BF16 = mybir.dt.bfloat16
I32 = mybir.dt.int32
BF = ml_dtypes.bfloat16

def _seg_ref(in0, in1, s0, s1, imm2):
    P, S_, N_ = in0.shape
    prod = (in0.astype(np.float32) * in1).reshape(P, S_, N_)
    return np.cumsum(prod, axis=-1).reshape(P, S_ * N_)


def _make_segmented(uops_1x):
    """[seed, steady] -> [seed, steady(subdim->step), step].

    The step state fires on SUB_DIM_DONE for exactly one element with the
    scan ADD stage reading the Zero lane instead of CURR_ALU_OUT, so the
    accumulator restarts at each 42-element page boundary."""
    seed, steady = uops_1x
    steady2 = dataclasses.replace(
        steady,
        trigger=(Trigger.SRC_TENSOR_DONE, Trigger.SUB_DIM_DONE, Trigger.NONE),
        next_uop=(0, 2, 0),
    )
    step_dp = [dataclasses.replace(d) for d in steady.datapath_config]
    step_dp[1] = dataclasses.replace(
        step_dp[1], alu_src0=AluInp.PREV_DELAY_2, alu_src1=AluInp.PREV_ALU_OUT
    )
    step = dataclasses.replace(
        steady,
        datapath_config=step_dp,
        trigger=(Trigger.SRC_TENSOR_DONE, Trigger.SUB_DIM_DONE, Trigger.COUNT),
        next_uop=(0, 2, 1),
        repeat_count=1,
    )
    return [seed, steady2, step]


def _dp2x(stage_over=None):
    """8-stage 2x_1port datapath: two packed bf16 elements per cycle.

    Lanes: L0=src0_lo L1=src1_lo L2=src0_hi L3=src1_hi L4=ZERO.
    s0 m0=w_lo*x_lo | s1 m1=w_hi*x_hi (cap m0->L0) | s2 p=m1+m0 (cap m1->L1)
    s3 acc=CURR+p (scan feedback) | s4 lo=acc-m1 (cap acc->L2) | s5-7 bypass.
    Out: WR0_LO=ALU_OUT (even prefix), WR0_HI=DELAY_2 (odd prefix = acc)."""
    P = AluInp.PREV_ALU_OUT
    plan = {
        0: (AluOp.MULTIPLY, AluInp.PREV_DELAY_0, AluInp.PREV_DELAY_1),
        1: (AluOp.MULTIPLY, AluInp.PREV_DELAY_2, AluInp.PREV_DELAY_3),
        2: (AluOp.ADD, P, AluInp.PREV_DELAY_0),
        3: (AluOp.ADD, AluInp.CURR_ALU_OUT, P),
        4: (AluOp.SUBTRACT, P, AluInp.PREV_DELAY_1),
        5: (AluOp.BYPASS, P, P),
        6: (AluOp.BYPASS, P, P),
        7: (AluOp.BYPASS, P, P),
    }
    if stage_over:
        plan.update(stage_over)
    caps = {1: 0, 2: 1, 4: 2}
    dps = []
    for s in range(8):
        op, a, b = plan[s]
        delay = [DelayInp.PREV_DELAY] * 7
        if s in caps:
            delay[caps[s]] = DelayInp.PREV_ALU_OUT
        dps.append(UopDpConfig(
            op=op, alu_src0=a, alu_src1=b, delay=delay,
            alu_out_enable=1, swap_enable=0,
            alu_out_a_enable=0, alu_out_b_enable=0,
            delay_enable=[1, 1, 1, 1, 1, 0, 0], idx0_sel=0, idx1_sel=0))
    return dps


def _uops_2x_segmented():
    inp = [InpSel.ZERO, InpSel.SRC_0, InpSel.SRC_1, InpSel.SRC_0_HI,
           InpSel.SRC_1_HI, InpSel.ZERO, InpSel.ZERO, InpSel.ZERO]
    inp_enable = [0, 1, 1, 1, 1, 1, 0, 0]
    out_off = {o: OutSel.ALU_OUT for o in OutPath}
    out_en_off = {o: DISABLE for o in OutPath}
    out_on = dict(out_off)
    out_on[OutPath.WR0_HI] = OutSel.DELAY_2
    out_en_on = dict(out_en_off)
    out_en_on[OutPath.WR0_LO] = ENABLE
    out_en_on[OutPath.WR0_HI] = ENABLE
    seed = UopConfig(
        datapath_config=_dp2x({3: (AluOp.BYPASS, AluInp.PREV_DELAY_4,
                                   AluInp.PREV_DELAY_4)}),
        inp=inp, inp_enable=inp_enable, out=out_off, out_enable=out_en_off,
        accum_enabled=DISABLE, require_inp0=0, require_inp1=0,
        trigger=(Trigger.COUNT, Trigger.NONE, Trigger.NONE),
        next_uop=(1, 0, 0), repeat_count=1)
    steady = UopConfig(
        datapath_config=_dp2x(),
        inp=inp, inp_enable=inp_enable, out=out_on, out_enable=out_en_on,
        accum_enabled=DISABLE, require_inp0=1, require_inp1=1,
        trigger=(Trigger.SRC_TENSOR_DONE, Trigger.SUB_DIM_DONE, Trigger.NONE),
        next_uop=(0, 2, 0), repeat_count=0)
    step = UopConfig(
        datapath_config=_dp2x({3: (AluOp.ADD, AluInp.PREV_DELAY_4,
                                   AluInp.PREV_ALU_OUT)}),
        inp=inp, inp_enable=inp_enable, out=out_on, out_enable=out_en_on,
        accum_enabled=DISABLE, require_inp0=1, require_inp1=1,
        trigger=(Trigger.SRC_TENSOR_DONE, Trigger.SUB_DIM_DONE, Trigger.COUNT),
        next_uop=(0, 2, 1), repeat_count=1)
    return [seed, steady, step]


def _register_seg_mac():
    name = "SEG_MAC_GNN"
    if name in dve_ops._SUB_OPCODE_FOR_NAME:
        return next(op for op in dve_ops.OPS if op.name == name)
    spec = Spec(body=scan(AluOp.ADD, Src0 * Src1), reference=_seg_ref)
    row = dve_ops._CUSTOM_DVE_ROW_BASE + len(dve_ops.OPS)
    shas = {}
    for ver in ("v3", "v4"):
        uops = _make_segmented(lower(spec, ver=ver))
        ospec = DveOpSpec(name=name, opcode=row, uops=uops,
                          uops_2x=_uops_2x_segmented(),
                          perf_max=1, rd1_en=_has_src1(spec))
        ospec.validate(ver)
        shas[ver] = ospec.sha(ver)
        dve_ops._COMPILE_CACHE[(name, ver)] = ospec
    op = dve_ops.DveOp(name, spec, subdim=True, uops_sha=shas,
                       perf_en={"v3": True, "v4": True})
    dve_ops.OPS.append(op)
    dve_ops._SUB_OPCODE_FOR_NAME[name] = row
    dve_ops.CUSTOM_DVE_SPECS[name] = spec
    return op


def _emit_seg_mac(nc, op, out, in0, in1):
    """nc.vector._custom_dve equivalent that sets perf_max=1 on the
    instruction so the engine may engage the 2x_1port table slot."""
    v = nc.vector
    if op.name not in v.bass.m.ant_custom_dve_ops:
        v.bass.m.ant_custom_dve_ops = sorted(
            {*v.bass.m.ant_custom_dve_ops, op.name})
    shape = bass_isa.CustomDveShape.STT
    isa_opcode = v.bass.isa.Opcode[
        f"NEURON_ISA_TPB_OPCODE_CUSTOM_DVE_ANT_{shape.slot()}"].value
    ins = [v.lower_ap(in0, for_isa=True, opt=False),
           v.lower_ap(in1, for_isa=True, opt=False),
           mybir.ImmediateValue(dtype=mybir.dt.float32, value=0.0),
           mybir.ImmediateValue(dtype=mybir.dt.float32, value=0.0)]
    outs = [v.lower_ap(out, for_isa=True, opt=False)]
    return v.add_instruction(
        bass_isa.InstCustomDveAnt(
            name=v.bass.get_next_instruction_name(),
            op_name=op.name, rd1_en=True, subdim=0x02, imm2=0.0,
            shape=shape, row=dve_ops.get_dve_sub_opcode(op.name),
            isa_opcode=isa_opcode, perf_max=1, ins=ins, outs=outs))


def _host_prep(n_feat, e_feat, src, dst):
    """Sort edges by dst, shard by dst block, pad each (core, window) edge run
    onto a shared slot grid so the tile->window map is identical on all cores."""
    order = np.argsort(dst, kind="stable")
    src_s, dst_s, ef_s = src[order], dst[order], e_feat[order]
    src_r = src_s   # cc_out rows are plain node ids (core-concat row-major)

    # per (core, window) counts
    core_e = dst_s // NPC
    loc = dst_s - core_e * NPC
    win_e = loc // WIN
    cnt = np.zeros((N_CORES, N_WIN), dtype=np.int64)
    np.add.at(cnt, (core_e, win_e), 1)

    slot_cnt = cnt.max(axis=0)                       # shared grid
    G = np.concatenate([[0], np.cumsum(slot_cnt)])   # window slot boundaries
    total = int(G[-1])
    T = (total + 127) // 128                         # edge tiles per core
    E_PAD = T * 128

    # per-core padded edge arrays
    src_pad = np.zeros((N_CORES, E_PAD), dtype=np.int32)
    dstrel_pad = np.full((N_CORES, E_PAD), -1.0, dtype=np.float32)
    ef_pad = np.zeros((N_CORES, E_PAD, E_IN), dtype=np.float32)

    # tile -> window band
    w0 = np.zeros(T, dtype=np.int64)       # first window overlapping tile t
    bw = np.zeros(T, dtype=np.int64)       # how many windows overlap tile t
    for t in range(T):
        lo, hi = t * 128, min((t + 1) * 128, total)
        wlo = int(np.searchsorted(G, lo, side="right") - 1)
        whi = int(np.searchsorted(G, max(hi - 1, lo), side="right") - 1)
        wlo, whi = min(wlo, N_WIN - 1), min(whi, N_WIN - 1)
        w0[t] = wlo
        bw[t] = whi - wlo + 1
    B_W = int(bw.max())

    # fill padded arrays: window w of core k occupies slots [G[w], G[w]+cnt[k,w])
    core_starts = np.searchsorted(core_e, np.arange(N_CORES))
    for k in range(N_CORES):
        base = core_starts[k]
        cw = np.concatenate([[0], np.cumsum(cnt[k])])
        for w in range(N_WIN):
            s0, s1 = int(base + cw[w]), int(base + cw[w + 1])
            g0 = int(G[w])
            n_e = s1 - s0
            src_pad[k, g0:g0 + n_e] = src_r[s0:s1]
            ef_pad[k, g0:g0 + n_e] = ef_s[s0:s1]
            # dst_rel relative to the band anchor of the edge's tile
            slots = np.arange(g0, g0 + n_e)
            dstrel_pad[k, g0:g0 + n_e] = (
                loc[s0:s1] - w0[slots // 128] * WIN).astype(np.float32)

    # scatter pair list (t, w) from actual overlap, and per-window tile ranges
    pairs = []
    for t in range(T):
        for j in range(int(bw[t])):
            w = int(w0[t]) + j
            if w < N_WIN:
                pairs.append((t, w))
    win_tiles = {w: [t for (t, ww) in pairs if ww == w] for w in range(N_WIN)}

    grid = dict(T=T, E_PAD=E_PAD, B_W=B_W, w0=w0, bw=bw, win_tiles=win_tiles)

    per_core = []
    for k in range(N_CORES):
        per_core.append(dict(
            e_featT=np.ascontiguousarray(ef_pad[k].T).astype(BF),  # [10, E_PAD]
            n_featT=np.ascontiguousarray(n_feat[k * NPC:(k + 1) * NPC].T),  # [42, NPC]
            src_idx=np.ascontiguousarray(src_pad[k].reshape(T, 128).T).astype(np.int32),  # [128, T]
            dst_rel=np.ascontiguousarray(dstrel_pad[k].reshape(T, 128).T),  # [128, T]
        ))
    return grid, per_core


def _build_program(grid):
    T, B_W = grid["T"], grid["B_W"]
    w0, bw, win_tiles = grid["w0"], grid["bw"], grid["win_tiles"]
    SEG_MAC = _register_seg_mac()

    # per-tile column offsets into the resident one-hot bank
    oh_off = np.zeros(T + 1, dtype=np.int64)
    for t in range(T):
        oh_off[t + 1] = oh_off[t] + int(bw[t]) * WIN
    OH_COLS = int(oh_off[T])

    nc = bacc.Bacc("TRN2", target_bir_lowering=False, debug=False,
                   num_devices=N_CORES)

    # ---- kernel I/O ----
    e_featT = nc.dram_tensor("e_featT", [E_IN, grid["E_PAD"]], BF16, kind="ExternalInput")
    n_featT = nc.dram_tensor("n_featT", [D_IN, NPC], F32, kind="ExternalInput")
    src_idx = nc.dram_tensor("src_idx", [128, T], I32, kind="ExternalInput")
    dst_rel = nc.dram_tensor("dst_rel", [128, T], F32, kind="ExternalInput")
    iota = nc.dram_tensor("iota", [128, B_W * WIN], BF16, kind="ExternalInput")
    en1_w = nc.dram_tensor("en1_w", [E_IN, E_H], BF16, kind="ExternalInput")
    en1_bc = nc.dram_tensor("en1_bc", [E_H, 1], F32, kind="ExternalInput")
    en2_wp = nc.dram_tensor("en2_wp", [E_H, NW], BF16, kind="ExternalInput")
    en2b_io = nc.dram_tensor("en2b_io", [D_H, D_H], BF16, kind="ExternalInput")
    lin0_wt = nc.dram_tensor("lin0_wt", [D_IN, D_H], F32, kind="ExternalInput")
    lin0_bc = nc.dram_tensor("lin0_bc", [D_H, 1], F32, kind="ExternalInput")
    msgw_top = nc.dram_tensor("msgw_top", [D_H, D_H], F32, kind="ExternalInput")
    msgw_bot = nc.dram_tensor("msgw_bot", [D_H, D_H], F32, kind="ExternalInput")
    msgb_c = nc.dram_tensor("msgb_c", [D_H, 1], F32, kind="ExternalInput")
    convb_c = nc.dram_tensor("convb_c", [D_H, 1], F32, kind="ExternalInput")
    ident = nc.dram_tensor("ident", [D_H, D_H], F32, kind="ExternalInput")
    y = nc.dram_tensor("y", [NPC, D_H], F32, kind="ExternalOutput")

    with tile.TileContext(nc) as tc:
        with (
            tc.tile_pool(name="const", bufs=1) as cpool,
            tc.tile_pool(name="dram", bufs=1, space="DRAM") as dram,
        ):
            # ---- persistent SBUF residents ----
            nfT_sb = cpool.tile([D_IN, NPC], F32)
            srci_sb = cpool.tile([128, T], I32)
            dstr_sb = cpool.tile([128, T], F32)
            iota_sb = cpool.tile([128, B_W * WIN], BF16)
            en1w_sb = cpool.tile([E_IN, E_H], BF16)
            en1bc_sb = cpool.tile([E_H, 1], F32)
            en2wp_sb = cpool.tile([E_H, NW], BF16)
            b_io_sb = cpool.tile([D_H, D_H], BF16)
            lin0w_sb = cpool.tile([D_IN, D_H], F32)
            lin0bc_sb = cpool.tile([D_H, 1], F32)
            mwt_sb = cpool.tile([D_H, D_H], F32)
            mwb_sb = cpool.tile([D_H, D_H], F32)
            mbc_sb = cpool.tile([D_H, 1], F32)
            cvbc_sb = cpool.tile([D_H, 1], F32)
            id_sb = cpool.tile([D_H, D_H], F32)
            outT_a = cpool.tile([D_H, NPC], F32)
            outT_b = cpool.tile([D_H, NPC], F32)
            oh_all = cpool.tile([128, OH_COLS], BF16)

            for sb, dr in [(nfT_sb, n_featT), (srci_sb, src_idx),
                           (dstr_sb, dst_rel), (iota_sb, iota), (en1w_sb, en1_w),
                           (en1bc_sb, en1_bc), (en2wp_sb, en2_wp), (b_io_sb, en2b_io),
                           (lin0w_sb, lin0_wt), (lin0bc_sb, lin0_bc), (mwt_sb, msgw_top),
                           (mwb_sb, msgw_bot), (mbc_sb, msgb_c), (cvbc_sb, convb_c),
                           (id_sb, ident)]:
                nc.sync.dma_start(sb[:], dr[:])

            # ---- DRAM scratch ----
            # W stored group-interleaved: group g row p holds tiles 4g..4g+3's
            # partition-p rows concatenated -> 14 KB contiguous per partition
            # per 1.8 MB group DMA (vs 3.5 KB runs at 451 KB granularity).
            G4 = (T + 3) // 4
            w_dram = dram.tile([G4 * 128, 4 * NW], BF16)
            cc_in = [dram.tile([NPC, D_H], BF16, name=f"cc_in{i}") for i in range(2)]
            cc_out = [dram.tile([N, D_H], BF16, name=f"cc_out{i}", addr_space="Shared")
                      for i in range(STEPS)]

            # ====== lin0 FIRST (quick), so AllGather 0 fires early and
            # ====== step-1 gathers stream during the W build ======
            with (
                tc.tile_pool(name="l0_sm", bufs=4) as l_sm,
                tc.tile_pool(name="l0_up", bufs=2, space="PSUM") as l_up,
                tc.tile_pool(name="l0_tr", bufs=1, space="PSUM") as l_tr,
            ):
                for w in range(N_WIN):
                    n0 = w * WIN
                    m = min(WIN, NPC - n0)
                    up = l_up.tile([D_H, WIN], F32, name="up")
                    nc.tensor.matmul(up[:, :m], lhsT=lin0w_sb[:],
                                     rhs=nfT_sb[:, n0:n0 + m],
                                     start=True, stop=True)
                    nc.scalar.activation(outT_a[:, n0:n0 + m], up[:, :m],
                                         mybir.ActivationFunctionType.Relu,
                                         bias=lin0bc_sb[:, 0:1])
                    tr = l_tr.tile([128, D_H], F32, name="tr")
                    nc.tensor.transpose(tr[:m, :], outT_a[:, n0:n0 + m], id_sb[:])
                    rows = l_sm.tile([128, D_H], BF16, name="rows_b")
                    nc.scalar.copy(rows[:m, :], tr[:m, :])
                    nc.sync.dma_start(cc_in[0][n0:n0 + m, :], rows[:m, :])
                nc.gpsimd.collective_compute(
                    "AllGather", mybir.AluOpType.bypass,
                    replica_groups=[list(range(N_CORES))],
                    ins=[cc_in[0].opt()], outs=[cc_out[0].opt()])

            # =========== setup: build W' (bf16, no bias) in HBM ===========
            ECH = 16   # e_feat tiles per SBUF chunk
            HB = 4     # en1 batch: 4 edge tiles per matmul / relu
            nsz = [512, 512, 512, NW - 3 * 512]
            noff = [0, 512, 1024, 1536]
            with (
                tc.tile_pool(name="su_h", bufs=2) as su_h,
                tc.tile_pool(name="su_sb", bufs=2) as su_sb,
                tc.tile_pool(name="su_e", bufs=2) as su_e,
                tc.tile_pool(name="su_ph", bufs=2, space="PSUM") as su_ph,
                tc.tile_pool(name="su_pw", bufs=3, space="PSUM") as su_pw,
            ):
                e_ch = None
                h_g = None
                w_sb = None
                for t in range(T):
                    if t % ECH == 0:
                        c0 = t * 128
                        c1 = min((t + ECH) * 128, grid["E_PAD"])
                        e_ch = su_e.tile([E_IN, ECH * 128], BF16, name="e_ch")
                        nc.sync.dma_start(e_ch[:, :c1 - c0], e_featT[:, c0:c1])
                    if t % HB == 0:
                        gm = min(HB, T - t) * 128
                        o = (t % ECH) * 128
                        ph = su_ph.tile([128, HB * 128], F32, name="ph")
                        nc.tensor.matmul(ph[:, :gm], lhsT=en1w_sb[:],
                                         rhs=e_ch[:, o:o + gm],
                                         start=True, stop=True)
                        h_g = su_h.tile([128, HB * 128], BF16, name="h_g")
                        nc.scalar.activation(h_g[:, :gm], ph[:, :gm],
                                             mybir.ActivationFunctionType.Relu,
                                             bias=en1bc_sb[:, 0:1])
                    h_t = h_g[:, (t % HB) * 128:(t % HB) * 128 + 128]
                    bwt = int(bw[t])
                    nc.vector.tensor_scalar(
                        out=oh_all[:, int(oh_off[t]):int(oh_off[t]) + bwt * WIN],
                        in0=iota_sb[:, :bwt * WIN],
                        scalar1=dstr_sb[:, t:t + 1],
                        scalar2=None, op0=mybir.AluOpType.is_equal)
                    if t % 4 == 0:
                        w_sb = su_sb.tile([128, 4 * NW], BF16, name="w_sb")
                    wo = (t % 4) * NW
                    for j in range(4):
                        o0 = noff[j]
                        pw = su_pw.tile([128, 512], F32, name="pw")
                        nc.tensor.matmul(pw[:, :nsz[j]], lhsT=h_t,
                                         rhs=en2wp_sb[:, o0:o0 + nsz[j]],
                                         start=True, stop=True)
                        # pure cast PSUM->SBUF: chunks 0-1 on Scalar, 2-3 on DVE
                        if j < 2:
                            nc.scalar.copy(w_sb[:, wo + o0:wo + o0 + nsz[j]],
                                           pw[:, :nsz[j]])
                        else:
                            nc.vector.tensor_scalar_add(
                                out=w_sb[:, wo + o0:wo + o0 + nsz[j]],
                                in0=pw[:, :nsz[j]], scalar1=0.0)
                    if t % 4 == 3 or t == T - 1:
                        g = t // 4
                        nc.sync.dma_start(
                            w_dram[g * 128:(g + 1) * 128, :wo + NW],
                            w_sb[:, :wo + NW])

            # =========== step pools ===========
            with (
                tc.tile_pool(name="st_w", bufs=4) as p_w,
                tc.tile_pool(name="st_x", bufs=28) as p_x,
                tc.tile_pool(name="st_m", bufs=8) as p_m,
                tc.tile_pool(name="st_sm", bufs=5) as p_sm,
                tc.tile_pool(name="ps_ag", bufs=3, space="PSUM") as ps_ag,
                tc.tile_pool(name="ps_xs", bufs=2, space="PSUM") as ps_xs,
                tc.tile_pool(name="ps_up", bufs=2, space="PSUM") as ps_up,
                tc.tile_pool(name="ps_tr", bufs=1, space="PSUM") as ps_tr,
            ):
                def window_cols(w):
                    n0 = w * WIN
                    m = min(WIN, NPC - n0)
                    return n0, m

                def update_window(w, outT_cur, outT_new, aggr_ps, xs_sb, step):
                    """Deferred window epilogue stages S1-S3 (emitted 1-3
                    tiles after the window's last scatter so the PE FIFO
                    never stalls on Scalar round-trips)."""
                    n0, m = window_cols(w)
                    last = step == STEPS

                    cell = {}

                    def s1():
                        # en2-bias contribution: aggr += B^T @ XS (bf16)
                        nc.tensor.matmul(aggr_ps[:, :m], lhsT=b_io_sb[:],
                                         rhs=xs_sb[:, :m], start=False, stop=False)
                        # + out (identity residual); conv bias folds into relu
                        nc.tensor.matmul(aggr_ps[:, :m], lhsT=id_sb[:],
                                         rhs=outT_cur[:, n0:n0 + m],
                                         start=False, stop=True)
                        mT_sb = p_sm.tile([D_H, WIN], F32, name="mT_sb")
                        nc.scalar.activation(mT_sb[:, :m], aggr_ps[:, :m],
                                             mybir.ActivationFunctionType.Relu,
                                             bias=cvbc_sb[:, 0:1])
                        cell["mT"] = mT_sb

                    def s2():
                        mT_sb = cell["mT"]
                        up = ps_up.tile([D_H, WIN], F32, name="up")
                        nc.tensor.matmul(up[:, :m], lhsT=mwt_sb[:], rhs=mT_sb[:, :m],
                                         start=True, stop=False)
                        nc.tensor.matmul(up[:, :m], lhsT=mwb_sb[:],
                                         rhs=outT_cur[:, n0:n0 + m],
                                         start=False, stop=not last)
                        if last:
                            nc.tensor.matmul(up[:, :m], lhsT=id_sb[:],
                                             rhs=nfT_sb[:, n0:n0 + m],
                                             start=False, stop=True)
                        # msg bias folds into the PSUM->SBUF copy
                        nc.scalar.activation(outT_new[:, n0:n0 + m], up[:, :m],
                                             mybir.ActivationFunctionType.Identity,
                                             bias=mbc_sb[:, 0:1])

                    def s3():
                        tr = ps_tr.tile([128, D_H], F32, name="tr")
                        nc.tensor.transpose(tr[:m, :], outT_new[:, n0:n0 + m], id_sb[:])
                        if last:
                            rows = p_sm.tile([128, D_H], F32, name="rows_f")
                            nc.scalar.copy(rows[:m, :], tr[:m, :])
                            nc.sync.dma_start(y[n0:n0 + m, :], rows[:m, :])
                        else:
                            rows = p_sm.tile([128, D_H], BF16, name="rows_b")
                            nc.scalar.copy(rows[:m, :], tr[:m, :])
                            nc.sync.dma_start(cc_in[step % 2][n0:n0 + m, :],
                                              rows[:m, :])
                    return [s1, s2, s3]

                def all_gather(step):
                    nc.gpsimd.collective_compute(
                        "AllGather", mybir.AluOpType.bypass,
                        replica_groups=[list(range(N_CORES))],
                        ins=[cc_in[step % 2].opt()], outs=[cc_out[step].opt()])

                # =========== message passing steps ===========
                for step in range(1, STEPS + 1):
                    outT_cur = outT_a if step % 2 == 1 else outT_b
                    outT_new = outT_b if step % 2 == 1 else outT_a
                    src_buf = cc_out[step - 1]
                    aggr_of = {}
                    xs_of = {}
                    pending = []        # (due_tile, stage_fn)
                    # gathers run LOOKAHEAD tiles ahead of the MAC loop so the
                    # Pool queue free-runs instead of pacing the pipeline
                    LOOKAHEAD = 24
                    xg_of = {}

                    def issue_gather(tt):
                        xg_of[tt] = p_x.tile([128, D_H], BF16, name="x_g")
                        nc.gpsimd.indirect_dma_start(
                            out=xg_of[tt][:], out_offset=None, in_=src_buf[:],
                            in_offset=bass.IndirectOffsetOnAxis(
                                ap=srci_sb[:, tt:tt + 1], axis=0))

                    for t in range(min(LOOKAHEAD, T)):
                        issue_gather(t)
                    for t in range(T):
                        if t + LOOKAHEAD < T:
                            issue_gather(t + LOOKAHEAD)
                        for due, fn in [p for p in pending if p[0] <= t]:
                            fn()
                        pending = [p for p in pending if p[0] > t]
                        x_g = xg_of.pop(t)[:]
                        g, gj = t // 4, t % 4
                        w_t = p_w.tile([128, NW], BF16, name="w_t")
                        nc.sync.dma_start(
                            w_t[:],
                            w_dram[g * 128:(g + 1) * 128, gj * NW:(gj + 1) * NW])
                        seg = p_m.tile([128, NW], BF16, name="seg")
                        _emit_seg_mac(
                            nc, SEG_MAC,
                            out=seg[:].rearrange("p (s n) -> p s n", s=D_H),
                            in0=w_t[:].rearrange("p (s n) -> p s n", s=D_H),
                            in1=x_g[:, None, :].to_broadcast([128, D_H, D_H]))
                        msg = seg[:, D_H - 1::D_H]      # [128, 42] segment ends
                        bwt = int(bw[t])
                        # scatter matmuls against the resident one-hot bank
                        for j in range(bwt):
                            w = int(w0[t]) + j
                            if w >= N_WIN:
                                continue
                            tiles_w = win_tiles[w]
                            if w not in aggr_of:
                                aggr_of[w] = ps_ag.tile([D_H, WIN], F32, name="aggr")
                                xs_of[w] = ps_xs.tile([D_H, WIN], F32, name="xs")
                            first = t == tiles_w[0]
                            last_t = t == tiles_w[-1]
                            o_c = int(oh_off[t]) + j * WIN
                            nc.tensor.matmul(aggr_of[w][:], lhsT=msg,
                                             rhs=oh_all[:, o_c:o_c + WIN],
                                             start=first, stop=False)
                            nc.tensor.matmul(xs_of[w][:], lhsT=x_g[:],
                                             rhs=oh_all[:, o_c:o_c + WIN],
                                             start=first, stop=last_t)
                            if last_t:
                                # S0: move XS out of PSUM now (Scalar), then
                                # defer the PE-bearing stages by 1-3 tiles
                                _, m_w = window_cols(w)
                                xs_sb = p_sm.tile([D_H, WIN], BF16, name="xs_sb")
                                nc.scalar.copy(xs_sb[:, :m_w],
                                               xs_of.pop(w)[:, :m_w])
                                stages = update_window(w, outT_cur, outT_new,
                                                       aggr_of.pop(w), xs_sb,
                                                       step)
                                pending.extend(
                                    (t + 1 + i, fn) for i, fn in enumerate(stages))
                    for due, fn in sorted(pending, key=lambda p: p[0]):
                        fn()
                    if step < STEPS:
                        all_gather(step)

    nc.compile()
    return nc


_CACHED = {}


def kernel(n_feat, e_feat, src, dst, lin0_w, lin0_b, en1_w, en1_b,
           en2_w, en2_b, conv_bias, msg_w, msg_b):
    n_feat = np.asarray(n_feat, dtype=np.float32)
    e_feat = np.asarray(e_feat, dtype=np.float32)
    src = np.asarray(src, dtype=np.int32)
    dst = np.asarray(dst, dtype=np.int32)

    grid, per_core = _host_prep(n_feat, e_feat, src, dst)

    key = (grid["T"], grid["B_W"], tuple(grid["w0"].tolist()))
    if key not in _CACHED:
        _CACHED.clear()
        _CACHED[key] = _build_program(grid)
    nc = _CACHED[key]

    en2_wp = np.ascontiguousarray(
        np.asarray(en2_w, np.float32).reshape(E_H, D_H, D_H).transpose(0, 2, 1).reshape(E_H, NW))
    shared = dict(
        iota=np.tile(np.arange(grid["B_W"] * WIN, dtype=np.float32), (128, 1)).astype(BF),
        en1_w=np.asarray(en1_w, np.float32).astype(BF),
        en1_bc=np.asarray(en1_b, np.float32).reshape(E_H, 1),
        en2_wp=en2_wp.astype(BF),
        en2b_io=np.ascontiguousarray(
            np.asarray(en2_b, np.float32).reshape(D_H, D_H)).astype(BF),
        lin0_wt=np.asarray(lin0_w, np.float32),
        lin0_bc=np.asarray(lin0_b, np.float32).reshape(D_H, 1),
        msgw_top=np.ascontiguousarray(np.asarray(msg_w, np.float32)[:D_H, :]),
        msgw_bot=np.ascontiguousarray(np.asarray(msg_w, np.float32)[D_H:, :]),
        msgb_c=np.asarray(msg_b, np.float32).reshape(D_H, 1),
        convb_c=np.asarray(conv_bias, np.float32).reshape(D_H, 1),
        ident=np.eye(D_H, dtype=np.float32),
    )
    in_maps = []
    for k in range(N_CORES):
        m = dict(shared)
        m.update(per_core[k])
        in_maps.append(m)

    res = bass_utils.run_bass_kernel_spmd(nc, in_maps, core_ids=list(range(N_CORES)))
    out = np.concatenate([res.results[k]["y"] for k in range(N_CORES)], axis=0)
    return out.astype(np.float32)


# revision 32
# speedup vs baseline: 1.0123x; 1.0123x over previous
"""Trainium2 Bass kernel for nn_GatherModel (NNConv GNN message passing).

8-core SPMD, edge-parallel sharded by destination node block:
  - core k owns nodes [k*6250, (k+1)*6250) and all edges whose dst lies there
  - per-edge weight matrices W'_e (o-major, WITHOUT the en2 bias) are built
    once on device (PE, bf16) and streamed bf16 from HBM each of the 6 steps
  - per-edge contraction msg = x_src @ W'_e runs on the Vector engine via a
    hand-built SEGMENTED multiply+scan DVE op (bf16 in/out, scan resets each
    42-element page) with a hand-written 2x_1port uop variant (two packed
    bf16 elements per cycle, ~1.07us per 128x1764 tile) — segment ends are
    the per-o sums, read directly as a strided lhsT by the scatter matmul
  - the en2 bias contribution is applied at the aggregate level:
    aggr_bias[o, v] = B^T @ XS where XS[i, v] = sum_e x[e, i] * onehot[e, v]
    is accumulated on the PE alongside the message scatter
  - scatter (segment-sum over dst) is a PE matmul against on-device-built
    bf16 one-hot window matrices; node update runs fp32 in transposed
    feature layout
  - each step ends with an 8-core AllGather of bf16 node features
"""
import dataclasses
import numpy as np
import ml_dtypes

import concourse.bacc as bacc
import concourse.bass as bass
import concourse.bass_isa as bass_isa
import concourse.mybir as mybir
import concourse.tile as tile
from concourse import bass_utils, dve_ops
from concourse.dve_spec import Spec, Src0, Src1, scan, AluOp, lower, _has_src1
from concourse.dve_uop import (DveOpSpec, Trigger, AluInp, DelayInp, InpSel,
                               OutSel, OutPath, UopConfig, UopDpConfig,
                               ENABLE, DISABLE)

N = 50000
E = 150000
D_IN = 42
D_H = 42
E_IN = 10
E_H = 128
STEPS = 6
N_CORES = 8
NPC = N // N_CORES          # 6250 nodes per core
WIN = 128                   # scatter window (node block) size
N_WIN = (NPC + WIN - 1) // WIN  # 49 windows per core, last partial (106)
NW = D_H * D_H              # 1764
F32 = mybir.dt.float32
BF16 = mybir.dt.bfloat16
I32 = mybir.dt.int32
BF = ml_dtypes.bfloat16

def _seg_ref(in0, in1, s0, s1, imm2):
    P, S_, N_ = in0.shape
    prod = (in0.astype(np.float32) * in1).reshape(P, S_, N_)
    return np.cumsum(prod, axis=-1).reshape(P, S_ * N_)


def _make_segmented(uops_1x):
    """[seed, steady] -> [seed, steady(subdim->step), step].

    The step state fires on SUB_DIM_DONE for exactly one element with the
    scan ADD stage reading the Zero lane instead of CURR_ALU_OUT, so the
    accumulator restarts at each 42-element page boundary."""
    seed, steady = uops_1x
    steady2 = dataclasses.replace(
        steady,
        trigger=(Trigger.SRC_TENSOR_DONE, Trigger.SUB_DIM_DONE, Trigger.NONE),
        next_uop=(0, 2, 0),
    )
    step_dp = [dataclasses.replace(d) for d in steady.datapath_config]
    step_dp[1] = dataclasses.replace(
        step_dp[1], alu_src0=AluInp.PREV_DELAY_2, alu_src1=AluInp.PREV_ALU_OUT
    )
    step = dataclasses.replace(
        steady,
        datapath_config=step_dp,
        trigger=(Trigger.SRC_TENSOR_DONE, Trigger.SUB_DIM_DONE, Trigger.COUNT),
        next_uop=(0, 2, 1),
        repeat_count=1,
    )
    return [seed, steady2, step]


def _dp2x(stage_over=None):
    """8-stage 2x_1port datapath: two packed bf16 elements per cycle.

    Lanes: L0=src0_lo L1=src1_lo L2=src0_hi L3=src1_hi L4=ZERO.
    s0 m0=w_lo*x_lo | s1 m1=w_hi*x_hi (cap m0->L0) | s2 p=m1+m0 (cap m1->L1)
    s3 acc=CURR+p (scan feedback) | s4 lo=acc-m1 (cap acc->L2) | s5-7 bypass.
    Out: WR0_LO=ALU_OUT (even prefix), WR0_HI=DELAY_2 (odd prefix = acc)."""
    P = AluInp.PREV_ALU_OUT
    plan = {
        0: (AluOp.MULTIPLY, AluInp.PREV_DELAY_0, AluInp.PREV_DELAY_1),
        1: (AluOp.MULTIPLY, AluInp.PREV_DELAY_2, AluInp.PREV_DELAY_3),
        2: (AluOp.ADD, P, AluInp.PREV_DELAY_0),
        3: (AluOp.ADD, AluInp.CURR_ALU_OUT, P),
        4: (AluOp.SUBTRACT, P, AluInp.PREV_DELAY_1),
        5: (AluOp.BYPASS, P, P),
        6: (AluOp.BYPASS, P, P),
        7: (AluOp.BYPASS, P, P),
    }
    if stage_over:
        plan.update(stage_over)
    caps = {1: 0, 2: 1, 4: 2}
    dps = []
    for s in range(8):
        op, a, b = plan[s]
        delay = [DelayInp.PREV_DELAY] * 7
        if s in caps:
            delay[caps[s]] = DelayInp.PREV_ALU_OUT
        dps.append(UopDpConfig(
            op=op, alu_src0=a, alu_src1=b, delay=delay,
            alu_out_enable=1, swap_enable=0,
            alu_out_a_enable=0, alu_out_b_enable=0,
            delay_enable=[1, 1, 1, 1, 1, 0, 0], idx0_sel=0, idx1_sel=0))
    return dps


def _uops_2x_segmented():
    inp = [InpSel.ZERO, InpSel.SRC_0, InpSel.SRC_1, InpSel.SRC_0_HI,
           InpSel.SRC_1_HI, InpSel.ZERO, InpSel.ZERO, InpSel.ZERO]
    inp_enable = [0, 1, 1, 1, 1, 1, 0, 0]
    out_off = {o: OutSel.ALU_OUT for o in OutPath}
    out_en_off = {o: DISABLE for o in OutPath}
    out_on = dict(out_off)
    out_on[OutPath.WR0_HI] = OutSel.DELAY_2
    out_en_on = dict(out_en_off)
    out_en_on[OutPath.WR0_LO] = ENABLE
    out_en_on[OutPath.WR0_HI] = ENABLE
    seed = UopConfig(
        datapath_config=_dp2x({3: (AluOp.BYPASS, AluInp.PREV_DELAY_4,
                                   AluInp.PREV_DELAY_4)}),
        inp=inp, inp_enable=inp_enable, out=out_off, out_enable=out_en_off,
        accum_enabled=DISABLE, require_inp0=0, require_inp1=0,
        trigger=(Trigger.COUNT, Trigger.NONE, Trigger.NONE),
        next_uop=(1, 0, 0), repeat_count=1)
    steady = UopConfig(
        datapath_config=_dp2x(),
        inp=inp, inp_enable=inp_enable, out=out_on, out_enable=out_en_on,
        accum_enabled=DISABLE, require_inp0=1, require_inp1=1,
        trigger=(Trigger.SRC_TENSOR_DONE, Trigger.SUB_DIM_DONE, Trigger.NONE),
        next_uop=(0, 2, 0), repeat_count=0)
    step = UopConfig(
        datapath_config=_dp2x({3: (AluOp.ADD, AluInp.PREV_DELAY_4,
                                   AluInp.PREV_ALU_OUT)}),
        inp=inp, inp_enable=inp_enable, out=out_on, out_enable=out_en_on,
        accum_enabled=DISABLE, require_inp0=1, require_inp1=1,
        trigger=(Trigger.SRC_TENSOR_DONE, Trigger.SUB_DIM_DONE, Trigger.COUNT),
        next_uop=(0, 2, 1), repeat_count=1)
    return [seed, steady, step]


def _register_seg_mac():
    name = "SEG_MAC_GNN"
    if name in dve_ops._SUB_OPCODE_FOR_NAME:
        return next(op for op in dve_ops.OPS if op.name == name)
    spec = Spec(body=scan(AluOp.ADD, Src0 * Src1), reference=_seg_ref)
    row = dve_ops._CUSTOM_DVE_ROW_BASE + len(dve_ops.OPS)
    shas = {}
    for ver in ("v3", "v4"):
        uops = _make_segmented(lower(spec, ver=ver))
        ospec = DveOpSpec(name=name, opcode=row, uops=uops,
                          uops_2x=_uops_2x_segmented(),
                          perf_max=1, rd1_en=_has_src1(spec))
        ospec.validate(ver)
        shas[ver] = ospec.sha(ver)
        dve_ops._COMPILE_CACHE[(name, ver)] = ospec
    op = dve_ops.DveOp(name, spec, subdim=True, uops_sha=shas,
                       perf_en={"v3": True, "v4": True})
    dve_ops.OPS.append(op)
    dve_ops._SUB_OPCODE_FOR_NAME[name] = row
    dve_ops.CUSTOM_DVE_SPECS[name] = spec
    return op


def _emit_seg_mac(nc, op, out, in0, in1):
    """nc.vector._custom_dve equivalent that sets perf_max=1 on the
    instruction so the engine may engage the 2x_1port table slot."""
    v = nc.vector
    if op.name not in v.bass.m.ant_custom_dve_ops:
        v.bass.m.ant_custom_dve_ops = sorted(
            {*v.bass.m.ant_custom_dve_ops, op.name})
    shape = bass_isa.CustomDveShape.STT
    isa_opcode = v.bass.isa.Opcode[
        f"NEURON_ISA_TPB_OPCODE_CUSTOM_DVE_ANT_{shape.slot()}"].value
    ins = [v.lower_ap(in0, for_isa=True, opt=False),
           v.lower_ap(in1, for_isa=True, opt=False),
           mybir.ImmediateValue(dtype=mybir.dt.float32, value=0.0),
           mybir.ImmediateValue(dtype=mybir.dt.float32, value=0.0)]
    outs = [v.lower_ap(out, for_isa=True, opt=False)]
    return v.add_instruction(
        bass_isa.InstCustomDveAnt(
            name=v.bass.get_next_instruction_name(),
            op_name=op.name, rd1_en=True, subdim=0x02, imm2=0.0,
            shape=shape, row=dve_ops.get_dve_sub_opcode(op.name),
            isa_opcode=isa_opcode, perf_max=1, ins=ins, outs=outs))


def _host_prep(n_feat, e_feat, src, dst):
    """Sort edges by dst, shard by dst block, pad each (core, window) edge run
    onto a shared slot grid so the tile->window map is identical on all cores."""
    order = np.argsort(dst, kind="stable")
    src_s, dst_s, ef_s = src[order], dst[order], e_feat[order]
    src_r = src_s   # cc_out rows are plain node ids (core-concat row-major)

    # per (core, window) counts
    core_e = dst_s // NPC
    loc = dst_s - core_e * NPC
    win_e = loc // WIN
    cnt = np.zeros((N_CORES, N_WIN), dtype=np.int64)
    np.add.at(cnt, (core_e, win_e), 1)

    slot_cnt = cnt.max(axis=0)                       # shared grid
    G = np.concatenate([[0], np.cumsum(slot_cnt)])   # window slot boundaries
    total = int(G[-1])
    T = (total + 127) // 128                         # edge tiles per core
    E_PAD = T * 128

    # per-core padded edge arrays
    src_pad = np.zeros((N_CORES, E_PAD), dtype=np.int32)
    dstrel_pad = np.full((N_CORES, E_PAD), -1.0, dtype=np.float32)
    ef_pad = np.zeros((N_CORES, E_PAD, E_IN), dtype=np.float32)

    # tile -> window band
    w0 = np.zeros(T, dtype=np.int64)       # first window overlapping tile t
    bw = np.zeros(T, dtype=np.int64)       # how many windows overlap tile t
    for t in range(T):
        lo, hi = t * 128, min((t + 1) * 128, total)
        wlo = int(np.searchsorted(G, lo, side="right") - 1)
        whi = int(np.searchsorted(G, max(hi - 1, lo), side="right") - 1)
        wlo, whi = min(wlo, N_WIN - 1), min(whi, N_WIN - 1)
        w0[t] = wlo
        bw[t] = whi - wlo + 1
    B_W = int(bw.max())

    # fill padded arrays: window w of core k occupies slots [G[w], G[w]+cnt[k,w])
    core_starts = np.searchsorted(core_e, np.arange(N_CORES))
    for k in range(N_CORES):
        base = core_starts[k]
        cw = np.concatenate([[0], np.cumsum(cnt[k])])
        for w in range(N_WIN):
            s0, s1 = int(base + cw[w]), int(base + cw[w + 1])
            g0 = int(G[w])
            n_e = s1 - s0
            src_pad[k, g0:g0 + n_e] = src_r[s0:s1]
            ef_pad[k, g0:g0 + n_e] = ef_s[s0:s1]
            # dst_rel relative to the band anchor of the edge's tile
            slots = np.arange(g0, g0 + n_e)
            dstrel_pad[k, g0:g0 + n_e] = (
                loc[s0:s1] - w0[slots // 128] * WIN).astype(np.float32)

    # scatter pair list (t, w) from actual overlap, and per-window tile ranges
    pairs = []
    for t in range(T):
        for j in range(int(bw[t])):
            w = int(w0[t]) + j
            if w < N_WIN:
                pairs.append((t, w))
    win_tiles = {w: [t for (t, ww) in pairs if ww == w] for w in range(N_WIN)}

    grid = dict(T=T, E_PAD=E_PAD, B_W=B_W, w0=w0, bw=bw, win_tiles=win_tiles)

    per_core = []
    for k in range(N_CORES):
        per_core.append(dict(
            e_featT=np.ascontiguousarray(ef_pad[k].T).astype(BF),  # [10, E_PAD]
            n_featT=np.ascontiguousarray(n_feat[k * NPC:(k + 1) * NPC].T),  # [42, NPC]
            src_idx=np.ascontiguousarray(src_pad[k].reshape(T, 128).T).astype(np.int32),  # [128, T]
            dst_rel=np.ascontiguousarray(dstrel_pad[k].reshape(T, 128).T),  # [128, T]
        ))
    return grid, per_core


def _build_program(grid):
    T, B_W = grid["T"], grid["B_W"]
    w0, bw, win_tiles = grid["w0"], grid["bw"], grid["win_tiles"]
    SEG_MAC = _register_seg_mac()

    # per-tile column offsets into the resident one-hot bank
    oh_off = np.zeros(T + 1, dtype=np.int64)
    for t in range(T):
        oh_off[t + 1] = oh_off[t] + int(bw[t]) * WIN
    OH_COLS = int(oh_off[T])

    nc = bacc.Bacc("TRN2", target_bir_lowering=False, debug=False,
                   num_devices=N_CORES)

    # ---- kernel I/O ----
    e_featT = nc.dram_tensor("e_featT", [E_IN, grid["E_PAD"]], BF16, kind="ExternalInput")
    n_featT = nc.dram_tensor("n_featT", [D_IN, NPC], F32, kind="ExternalInput")
    src_idx = nc.dram_tensor("src_idx", [128, T], I32, kind="ExternalInput")
    dst_rel = nc.dram_tensor("dst_rel", [128, T], F32, kind="ExternalInput")
    iota = nc.dram_tensor("iota", [128, B_W * WIN], BF16, kind="ExternalInput")
    en1_w = nc.dram_tensor("en1_w", [E_IN, E_H], BF16, kind="ExternalInput")
    en1_bc = nc.dram_tensor("en1_bc", [E_H, 1], F32, kind="ExternalInput")
    en2_wp = nc.dram_tensor("en2_wp", [E_H, NW], BF16, kind="ExternalInput")
    en2b_io = nc.dram_tensor("en2b_io", [D_H, D_H], BF16, kind="ExternalInput")
    lin0_wt = nc.dram_tensor("lin0_wt", [D_IN, D_H], F32, kind="ExternalInput")
    lin0_bc = nc.dram_tensor("lin0_bc", [D_H, 1], F32, kind="ExternalInput")
    msgw_top = nc.dram_tensor("msgw_top", [D_H, D_H], F32, kind="ExternalInput")
    msgw_bot = nc.dram_tensor("msgw_bot", [D_H, D_H], F32, kind="ExternalInput")
    msgb_c = nc.dram_tensor("msgb_c", [D_H, 1], F32, kind="ExternalInput")
    convb_c = nc.dram_tensor("convb_c", [D_H, 1], F32, kind="ExternalInput")
    ident = nc.dram_tensor("ident", [D_H, D_H], F32, kind="ExternalInput")
    y = nc.dram_tensor("y", [NPC, D_H], F32, kind="ExternalOutput")

    with tile.TileContext(nc) as tc:
        with (
            tc.tile_pool(name="const", bufs=1) as cpool,
            tc.tile_pool(name="dram", bufs=1, space="DRAM") as dram,
        ):
            # ---- persistent SBUF residents ----
            nfT_sb = cpool.tile([D_IN, NPC], F32)
            srci_sb = cpool.tile([128, T], I32)
            dstr_sb = cpool.tile([128, T], F32)
            iota_sb = cpool.tile([128, B_W * WIN], BF16)
            en1w_sb = cpool.tile([E_IN, E_H], BF16)
            en1bc_sb = cpool.tile([E_H, 1], F32)
            en2wp_sb = cpool.tile([E_H, NW], BF16)
            b_io_sb = cpool.tile([D_H, D_H], BF16)
            lin0w_sb = cpool.tile([D_IN, D_H], F32)
            lin0bc_sb = cpool.tile([D_H, 1], F32)
            mwt_sb = cpool.tile([D_H, D_H], F32)
            mwb_sb = cpool.tile([D_H, D_H], F32)
            mbc_sb = cpool.tile([D_H, 1], F32)
            cvbc_sb = cpool.tile([D_H, 1], F32)
            id_sb = cpool.tile([D_H, D_H], F32)
            outT_a = cpool.tile([D_H, NPC], F32)
            outT_b = cpool.tile([D_H, NPC], F32)
            oh_all = cpool.tile([128, OH_COLS], BF16)

            for sb, dr in [(nfT_sb, n_featT), (srci_sb, src_idx),
                           (dstr_sb, dst_rel), (iota_sb, iota), (en1w_sb, en1_w),
                           (en1bc_sb, en1_bc), (en2wp_sb, en2_wp), (b_io_sb, en2b_io),
                           (lin0w_sb, lin0_wt), (lin0bc_sb, lin0_bc), (mwt_sb, msgw_top),
                           (mwb_sb, msgw_bot), (mbc_sb, msgb_c), (cvbc_sb, convb_c),
                           (id_sb, ident)]:
                nc.sync.dma_start(sb[:], dr[:])

            # ---- DRAM scratch ----
            # W stored group-interleaved: group g row p holds tiles 4g..4g+3's
            # partition-p rows concatenated -> 14 KB contiguous per partition
            # per 1.8 MB group DMA (vs 3.5 KB runs at 451 KB granularity).
            G4 = (T + 3) // 4
            w_dram = dram.tile([G4 * 128, 4 * NW], BF16)
            cc_in = [dram.tile([NPC, D_H], BF16, name=f"cc_in{i}") for i in range(2)]
            cc_out = [dram.tile([N, D_H], BF16, name=f"cc_out{i}", addr_space="Shared")
                      for i in range(STEPS)]

            # ====== lin0 FIRST (quick), so AllGather 0 fires early and
            # ====== step-1 gathers stream during the W build ======
            with (
                tc.tile_pool(name="l0_sm", bufs=4) as l_sm,
                tc.tile_pool(name="l0_up", bufs=2, space="PSUM") as l_up,
                tc.tile_pool(name="l0_tr", bufs=1, space="PSUM") as l_tr,
            ):
                for w in range(N_WIN):
                    n0 = w * WIN
                    m = min(WIN, NPC - n0)
                    up = l_up.tile([D_H, WIN], F32, name="up")
                    nc.tensor.matmul(up[:, :m], lhsT=lin0w_sb[:],
                                     rhs=nfT_sb[:, n0:n0 + m],
                                     start=True, stop=True)
                    nc.scalar.activation(outT_a[:, n0:n0 + m], up[:, :m],
                                         mybir.ActivationFunctionType.Relu,
                                         bias=lin0bc_sb[:, 0:1])
                    tr = l_tr.tile([128, D_H], F32, name="tr")
                    nc.tensor.transpose(tr[:m, :], outT_a[:, n0:n0 + m], id_sb[:])
                    rows = l_sm.tile([128, D_H], BF16, name="rows_b")
                    nc.scalar.copy(rows[:m, :], tr[:m, :])
                    nc.sync.dma_start(cc_in[0][n0:n0 + m, :], rows[:m, :])
                nc.gpsimd.collective_compute(
                    "AllGather", mybir.AluOpType.bypass,
                    replica_groups=[list(range(N_CORES))],
                    ins=[cc_in[0].opt()], outs=[cc_out[0].opt()])

            # =========== setup: build W' (bf16, no bias) in HBM ===========
            ECH = 16   # e_feat tiles per SBUF chunk
            HB = 4     # en1 batch: 4 edge tiles per matmul / relu
            nsz = [512, 512, 512, NW - 3 * 512]
            noff = [0, 512, 1024, 1536]
            with (
                tc.tile_pool(name="su_h", bufs=2) as su_h,
                tc.tile_pool(name="su_sb", bufs=2) as su_sb,
                tc.tile_pool(name="su_e", bufs=2) as su_e,
                tc.tile_pool(name="su_ph", bufs=2, space="PSUM") as su_ph,
                tc.tile_pool(name="su_pw", bufs=3, space="PSUM") as su_pw,
            ):
                e_ch = None
                h_g = None
                w_sb = None
                for t in range(T):
                    if t % ECH == 0:
                        c0 = t * 128
                        c1 = min((t + ECH) * 128, grid["E_PAD"])
                        e_ch = su_e.tile([E_IN, ECH * 128], BF16, name="e_ch")
                        nc.sync.dma_start(e_ch[:, :c1 - c0], e_featT[:, c0:c1])
                    if t % HB == 0:
                        gm = min(HB, T - t) * 128
                        o = (t % ECH) * 128
                        ph = su_ph.tile([128, HB * 128], F32, name="ph")
                        nc.tensor.matmul(ph[:, :gm], lhsT=en1w_sb[:],
                                         rhs=e_ch[:, o:o + gm],
                                         start=True, stop=True)
                        h_g = su_h.tile([128, HB * 128], BF16, name="h_g")
                        nc.scalar.activation(h_g[:, :gm], ph[:, :gm],
                                             mybir.ActivationFunctionType.Relu,
                                             bias=en1bc_sb[:, 0:1])
                    h_t = h_g[:, (t % HB) * 128:(t % HB) * 128 + 128]
                    bwt = int(bw[t])
                    nc.vector.tensor_scalar(
                        out=oh_all[:, int(oh_off[t]):int(oh_off[t]) + bwt * WIN],
                        in0=iota_sb[:, :bwt * WIN],
                        scalar1=dstr_sb[:, t:t + 1],
                        scalar2=None, op0=mybir.AluOpType.is_equal)
                    if t % 4 == 0:
                        w_sb = su_sb.tile([128, 4 * NW], BF16, name="w_sb")
                    wo = (t % 4) * NW
                    for j in range(4):
                        o0 = noff[j]
                        pw = su_pw.tile([128, 512], F32, name="pw")
                        nc.tensor.matmul(pw[:, :nsz[j]], lhsT=h_t,
                                         rhs=en2wp_sb[:, o0:o0 + nsz[j]],
                                         start=True, stop=True)
                        # pure cast PSUM->SBUF: chunks 0-1 on Scalar, 2-3 on DVE
                        if j < 2:
                            nc.scalar.copy(w_sb[:, wo + o0:wo + o0 + nsz[j]],
                                           pw[:, :nsz[j]])
                        else:
                            nc.vector.tensor_scalar_add(
                                out=w_sb[:, wo + o0:wo + o0 + nsz[j]],
                                in0=pw[:, :nsz[j]], scalar1=0.0)
                    if t % 4 == 3 or t == T - 1:
                        g = t // 4
                        nc.sync.dma_start(
                            w_dram[g * 128:(g + 1) * 128, :wo + NW],
                            w_sb[:, :wo + NW])

            # =========== step pools ===========
            with (
                tc.tile_pool(name="st_w", bufs=5) as p_w,
                tc.tile_pool(name="st_x", bufs=28) as p_x,
                tc.tile_pool(name="st_m", bufs=6) as p_m,
                tc.tile_pool(name="st_sm", bufs=5) as p_sm,
                tc.tile_pool(name="ps_ag", bufs=3, space="PSUM") as ps_ag,
                tc.tile_pool(name="ps_xs", bufs=2, space="PSUM") as ps_xs,
                tc.tile_pool(name="ps_up", bufs=2, space="PSUM") as ps_up,
                tc.tile_pool(name="ps_tr", bufs=1, space="PSUM") as ps_tr,
            ):
                def window_cols(w):
                    n0 = w * WIN
                    m = min(WIN, NPC - n0)
                    return n0, m

                def update_window(w, outT_cur, outT_new, aggr_ps, xs_sb, step):
                    """Deferred window epilogue stages S1-S3 (emitted 1-3
                    tiles after the window's last scatter so the PE FIFO
                    never stalls on Scalar round-trips)."""
                    n0, m = window_cols(w)
                    last = step == STEPS

                    cell = {}

                    def s1():
                        # en2-bias contribution: aggr += B^T @ XS (bf16)
                        nc.tensor.matmul(aggr_ps[:, :m], lhsT=b_io_sb[:],
                                         rhs=xs_sb[:, :m], start=False, stop=False)
                        # + out (identity residual); conv bias folds into relu
                        nc.tensor.matmul(aggr_ps[:, :m], lhsT=id_sb[:],
                                         rhs=outT_cur[:, n0:n0 + m],
                                         start=False, stop=True)
                        mT_sb = p_sm.tile([D_H, WIN], F32, name="mT_sb")
                        nc.scalar.activation(mT_sb[:, :m], aggr_ps[:, :m],
                                             mybir.ActivationFunctionType.Relu,
                                             bias=cvbc_sb[:, 0:1])
                        cell["mT"] = mT_sb

                    def s2():
                        mT_sb = cell["mT"]
                        up = ps_up.tile([D_H, WIN], F32, name="up")
                        nc.tensor.matmul(up[:, :m], lhsT=mwt_sb[:], rhs=mT_sb[:, :m],
                                         start=True, stop=False)
                        nc.tensor.matmul(up[:, :m], lhsT=mwb_sb[:],
                                         rhs=outT_cur[:, n0:n0 + m],
                                         start=False, stop=not last)
                        if last:
                            nc.tensor.matmul(up[:, :m], lhsT=id_sb[:],
                                             rhs=nfT_sb[:, n0:n0 + m],
                                             start=False, stop=True)
                        # msg bias folds into the PSUM->SBUF copy
                        nc.scalar.activation(outT_new[:, n0:n0 + m], up[:, :m],
                                             mybir.ActivationFunctionType.Identity,
                                             bias=mbc_sb[:, 0:1])

                    def s3():
                        tr = ps_tr.tile([128, D_H], F32, name="tr")
                        nc.tensor.transpose(tr[:m, :], outT_new[:, n0:n0 + m], id_sb[:])
                        if last:
                            rows = p_sm.tile([128, D_H], F32, name="rows_f")
                            nc.scalar.copy(rows[:m, :], tr[:m, :])
                            nc.sync.dma_start(y[n0:n0 + m, :], rows[:m, :])
                        else:
                            rows = p_sm.tile([128, D_H], BF16, name="rows_b")
                            nc.scalar.copy(rows[:m, :], tr[:m, :])
                            nc.sync.dma_start(cc_in[step % 2][n0:n0 + m, :],
                                              rows[:m, :])
                    return [s1, s2, s3]

                def all_gather(step):
                    nc.gpsimd.collective_compute(
                        "AllGather", mybir.AluOpType.bypass,
                        replica_groups=[list(range(N_CORES))],
                        ins=[cc_in[step % 2].opt()], outs=[cc_out[step].opt()])

                # =========== message passing steps ===========
                for step in range(1, STEPS + 1):
                    outT_cur = outT_a if step % 2 == 1 else outT_b
                    outT_new = outT_b if step % 2 == 1 else outT_a
                    src_buf = cc_out[step - 1]
                    aggr_of = {}
                    xs_of = {}
                    pending = []        # (due_tile, stage_fn)
                    # gathers run LOOKAHEAD tiles ahead of the MAC loop so the
                    # Pool queue free-runs instead of pacing the pipeline
                    LOOKAHEAD = 24
                    xg_of = {}

                    def issue_gather(tt):
                        xg_of[tt] = p_x.tile([128, D_H], BF16, name="x_g")
                        nc.gpsimd.indirect_dma_start(
                            out=xg_of[tt][:], out_offset=None, in_=src_buf[:],
                            in_offset=bass.IndirectOffsetOnAxis(
                                ap=srci_sb[:, tt:tt + 1], axis=0))

                    for t in range(min(LOOKAHEAD, T)):
                        issue_gather(t)
                    for t in range(T):
                        if t + LOOKAHEAD < T:
                            issue_gather(t + LOOKAHEAD)
                        for due, fn in [p for p in pending if p[0] <= t]:
                            fn()
                        pending = [p for p in pending if p[0] > t]
                        x_g = xg_of.pop(t)[:]
                        g, gj = t // 4, t % 4
                        w_t = p_w.tile([128, NW], BF16, name="w_t")
                        nc.sync.dma_start(
                            w_t[:],
                            w_dram[g * 128:(g + 1) * 128, gj * NW:(gj + 1) * NW])
                        seg = p_m.tile([128, NW], BF16, name="seg")
                        _emit_seg_mac(
                            nc, SEG_MAC,
                            out=seg[:].rearrange("p (s n) -> p s n", s=D_H),
                            in0=w_t[:].rearrange("p (s n) -> p s n", s=D_H),
                            in1=x_g[:, None, :].to_broadcast([128, D_H, D_H]))
                        msg = seg[:, D_H - 1::D_H]      # [128, 42] segment ends
                        bwt = int(bw[t])
                        # scatter matmuls against the resident one-hot bank
                        for j in range(bwt):
                            w = int(w0[t]) + j
                            if w >= N_WIN:
                                continue
                            tiles_w = win_tiles[w]
                            if w not in aggr_of:
                                aggr_of[w] = ps_ag.tile([D_H, WIN], F32, name="aggr")
                                xs_of[w] = ps_xs.tile([D_H, WIN], F32, name="xs")
                            first = t == tiles_w[0]
                            last_t = t == tiles_w[-1]
                            o_c = int(oh_off[t]) + j * WIN
                            nc.tensor.matmul(aggr_of[w][:], lhsT=msg,
                                             rhs=oh_all[:, o_c:o_c + WIN],
                                             start=first, stop=False)
                            nc.tensor.matmul(xs_of[w][:], lhsT=x_g[:],
                                             rhs=oh_all[:, o_c:o_c + WIN],
                                             start=first, stop=last_t)
                            if last_t:
                                # S0: move XS out of PSUM now (Scalar), then
                                # defer the PE-bearing stages by 1-3 tiles
                                _, m_w = window_cols(w)
                                xs_sb = p_sm.tile([D_H, WIN], BF16, name="xs_sb")
                                nc.scalar.copy(xs_sb[:, :m_w],
                                               xs_of.pop(w)[:, :m_w])
                                stages = update_window(w, outT_cur, outT_new,
                                                       aggr_of.pop(w), xs_sb,
                                                       step)
                                pending.extend(
                                    (t + 1 + i, fn) for i, fn in enumerate(stages))
                    for due, fn in sorted(pending, key=lambda p: p[0]):
                        fn()
                    if step < STEPS:
                        all_gather(step)

    nc.compile()
    return nc


_CACHED = {}


def kernel(n_feat, e_feat, src, dst, lin0_w, lin0_b, en1_w, en1_b,
           en2_w, en2_b, conv_bias, msg_w, msg_b):
    n_feat = np.asarray(n_feat, dtype=np.float32)
    e_feat = np.asarray(e_feat, dtype=np.float32)
    src = np.asarray(src, dtype=np.int32)
    dst = np.asarray(dst, dtype=np.int32)

    grid, per_core = _host_prep(n_feat, e_feat, src, dst)

    key = (grid["T"], grid["B_W"], tuple(grid["w0"].tolist()))
    if key not in _CACHED:
        _CACHED.clear()
        _CACHED[key] = _build_program(grid)
    nc = _CACHED[key]

    en2_wp = np.ascontiguousarray(
        np.asarray(en2_w, np.float32).reshape(E_H, D_H, D_H).transpose(0, 2, 1).reshape(E_H, NW))
    shared = dict(
        iota=np.tile(np.arange(grid["B_W"] * WIN, dtype=np.float32), (128, 1)).astype(BF),
        en1_w=np.asarray(en1_w, np.float32).astype(BF),
        en1_bc=np.asarray(en1_b, np.float32).reshape(E_H, 1),
        en2_wp=en2_wp.astype(BF),
        en2b_io=np.ascontiguousarray(
            np.asarray(en2_b, np.float32).reshape(D_H, D_H)).astype(BF),
        lin0_wt=np.asarray(lin0_w, np.float32),
        lin0_bc=np.asarray(lin0_b, np.float32).reshape(D_H, 1),
        msgw_top=np.ascontiguousarray(np.asarray(msg_w, np.float32)[:D_H, :]),
        msgw_bot=np.ascontiguousarray(np.asarray(msg_w, np.float32)[D_H:, :]),
        msgb_c=np.asarray(msg_b, np.float32).reshape(D_H, 1),
        convb_c=np.asarray(conv_bias, np.float32).reshape(D_H, 1),
        ident=np.eye(D_H, dtype=np.float32),
    )
    in_maps = []
    for k in range(N_CORES):
        m = dict(shared)
        m.update(per_core[k])
        in_maps.append(m)

    res = bass_utils.run_bass_kernel_spmd(nc, in_maps, core_ids=list(range(N_CORES)))
    out = np.concatenate([res.results[k]["y"] for k in range(N_CORES)], axis=0)
    return out.astype(np.float32)
